# revision 15
# baseline (speedup 1.0000x reference)
"""DCT-feature-extractor kernel for 8 Trainium2 NeuronCores (bf16 streams).

Math collapse: the reference keeps only dct[0, 0:4] of each 8x8 block's 2-D
orthonormal-DFT real part.  Row 0 of the DFT matrix is constant, so

    feat[m] = sum_l G[m, l] * colsum[l],   G[m, l] = cos(2*pi*m*l/8) / 8,

where colsum[l] is the column sum of the 8x8 block, followed by the linear
layer out = feats @ W^T + bias.

Sharding: split the 512 image rows (block-row groups i) and the matching
weight columns across 8 cores; the host sums the 8 [32, 512] partials + bias.

v2 changes vs the 39us baseline:
  * x and W stream as bf16 (tolerance is 2e-2; bf16 lands ~2e-3), halving
    the 8.6 MB/core of HBM traffic that paced the run.
  * x is host-transposed to [q4, w128, (a8, b32, i8)] so the DVE column-sum
    tree directly yields yT[w, (b, i)] per 128-column quarter -- the PE
    transposes, their PSUM->SBUF copies, and the 64 KB identity const are
    gone, shortening the x -> feats dependency chain.
  * The TileContext exit barrier rounds (InstDrain + barrier_* semaphores,
    ~1.5us serialized 5-engine round-robins each) are stripped from the end
    block; only the SP waits on the DMA-completion semaphores remain, so the
    output DMA still gates NEFF completion.
"""

import numpy as np
import ml_dtypes

import concourse.bacc as bacc
import concourse.mybir as mybir
from concourse.bass_utils import run_bass_kernel_spmd
from concourse.tile import TileContext

N_CORES = 8
B = 32            # batch
H = 512           # image height
WD = 512          # image width
BS = 8            # dct block size
NF = 4            # kept dct coefficients per block
OUT = 512         # linear output dim
RPC = H // N_CORES          # 64 rows per core
IPC = RPC // BS             # 8 block-rows per core
F32 = mybir.dt.float32
BF16 = mybir.dt.bfloat16
NPBF16 = ml_dtypes.bfloat16

N_WT_CHUNKS = 4   # weight streamed in 4 chunks of 4 output-tiles each
                  # (4 KB per-partition descriptors; 2 KB ones run ~30% slower)
NT = 2 * IPC      # 16 weight tiles of [128, 512]


def _g_mat():
    m = np.arange(NF)[:, None].astype(np.float64)
    l = np.arange(BS)[None, :].astype(np.float64)
    return (np.cos(2.0 * np.pi * m * l / BS) / 8.0).astype(np.float32)  # [4, 8]


def _consts():
    """[128, 256] bf16 = G_lo | G_hi.

    G_*[p=(j16,l8), q=(wc2,j16',m4)] = G[m, l] * (j16 == j16'), 'lo' filling
    q < 64 and 'hi' q >= 64, so two accumulating matmuls (rhs = yT of quarter
    2*fc, 2*fc+1) yield the [128, 256] featsT tile for w-half fc.
    """
    g = _g_mat()
    block = np.zeros((128, 64), np.float32)
    for j in range(16):
        block[j * 8:(j + 1) * 8, j * 4:(j + 1) * 4] = g.T  # [l, m]
    c = np.zeros((128, 256), np.float32)
    c[:, 0:64] = block      # G_lo: M cols 0..63  (wc2 = 0)
    c[:, 192:256] = block   # G_hi: M cols 64..127 (wc2 = 1)
    return c.astype(NPBF16)


def _build_bass():
    nc = bacc.Bacc("TRN2", target_bir_lowering=False, debug=False)
    # Strip the Bass.__init__ entry barrier (drain + event-sem per engine):
    # it only guards framework const-AP memsets this kernel never reads, and
    # it stalls the DMA queues ~4us behind the slow-to-start Tensor engine.
    entry = nc.main_func.blocks[0]
    for inst in [
        i for i in entry.instructions
        if isinstance(i, (mybir.InstDrain, mybir.InstEventSemaphore))
    ]:
        entry.instructions.remove(inst)
    # x host-prepped: [q4, p=w128, f=(a8, b32, i8)] bf16
    x = nc.dram_tensor("x", [4, 128, BS * B * IPC], BF16, kind="ExternalInput")
    # cst host-prepped: [128, G_lo 128 | G_hi 128] bf16
    cst = nc.dram_tensor("cst", [128, 256], BF16, kind="ExternalInput")
    # wt host-prepped: [p=(wc2,j16,m4), f=(fc,i) x o] bf16
    wt = nc.dram_tensor("wt", [128, NT * OUT], BF16, kind="ExternalInput")
    # 4 un-collapsed PSUM column-group partials; the host sums them
    out = nc.dram_tensor("out", [4 * B, OUT], F32, kind="ExternalOutput")

    with TileContext(nc) as tc:
        with (
            tc.tile_pool(name="sb", bufs=1) as sb,
            tc.tile_pool(name="ps", bufs=1, space="PSUM") as ps,
        ):
            # ---- DMA program order == HWDGE FIFO order per queue ----
            # Sync: x quarters then weight chunks (the stage-3 matmuls chase
            # the arriving chunks); Scalar concurrently: consts, later out.
            csts = sb.tile([128, 256], BF16, tag="cst")
            xt = [
                sb.tile([128, BS * B * IPC], BF16, tag=f"x{q}", name=f"x{q}")
                for q in range(4)
            ]
            # Scalar issues quarter 0 (its descriptor-gen overlaps Sync's
            # for quarter 1, starting the stream earlier), then consts.
            nc.scalar.dma_start(out=xt[0][:, :], in_=x.ap()[0])
            for q in range(1, 4):
                nc.sync.dma_start(out=xt[q][:, :], in_=x.ap()[q])
            nc.scalar.dma_start(out=csts[:, :], in_=cst.ap())
            glo, ghi = csts[:, 0:128], csts[:, 128:256]
            wts = sb.tile([128, NT * OUT], BF16, tag="wt")
            wck = NT * OUT // N_WT_CHUNKS
            for k in range(N_WT_CHUNKS):
                nc.sync.dma_start(
                    out=wts[:, k * wck:(k + 1) * wck],
                    in_=wt.ap()[:, k * wck:(k + 1) * wck],
                )

            # ---- stage 1: column-sum tree (DVE) per quarter; the host
            # layout f=(a8, b32, i8) makes each level contiguous and the
            # result IS yT[w, (b, i)] -- no transpose needed ----
            yts = []
            for q in range(4):
                t = xt[q]
                nc.vector.tensor_add(t[:, 0:1024], t[:, 0:1024], t[:, 1024:2048])
                nc.vector.tensor_add(t[:, 0:512], t[:, 0:512], t[:, 512:1024])
                yt = sb.tile([128, 256], BF16, tag=f"yt{q}")
                nc.vector.tensor_add(yt[:, :], t[:, 0:256], t[:, 256:512])
                yts.append(yt)

            # ---- stage 2 per w-half fc: block-diag G matmuls ----
            fts = []
            for fc in range(2):
                pft = ps.tile([128, 256], F32, tag=f"pft{fc}")
                nc.tensor.matmul(pft[:, :], glo, yts[2 * fc][:, :], start=True, stop=False)
                nc.tensor.matmul(pft[:, :], ghi, yts[2 * fc + 1][:, :], start=False, stop=True)
                ft = sb.tile([128, 256], BF16, tag=f"ft{fc}")
                nc.vector.tensor_copy(ft[:, :], pft[:, :])
                fts.append(ft)

            # ---- stage 3: 16 accumulating matmuls spread over the 4 PE
            # column groups (out partition offset 32*g -> tile_position), so
            # weight loads of one group overlap matmuls of another ----
            pout = ps.tile([128, OUT], F32, tag="pout")
            for fc in range(2):
                for i in range(IPC):
                    t = fc * IPC + i
                    g = t % 4
                    nc.tensor.matmul(
                        pout[32 * g:32 * (g + 1), :],
                        fts[fc][:, i::IPC],
                        wts[:, t * OUT:(t + 1) * OUT],
                        start=(t < 4),
                        stop=(t >= NT - 4),
                        tile_position=(0, 32 * g),
                        skip_group_check=True,
                    )
            # Ship the 4 col-group partials; the host sums them -- drops the
            # on-device CAST + collapse matmul + second copy (~2us of
            # serialized tail after the last weight chunk).  DMA cannot read
            # PSUM, so one f32 copy to SBUF remains, split across DVE and
            # the otherwise-idle GpSimd so the halves run in parallel.
            outs = sb.tile([128, OUT], F32, tag="outs")
            nc.vector.tensor_copy(outs[:, :], pout[:, :])
            nc.scalar.dma_start(out=out.ap(), in_=outs[:, :])

    # ---- strip the exit barrier rounds: keep only the SP event-semaphore
    # waits on the DMA-completion semaphores (they gate the out DMA); the
    # drains + barrier_* round-robins only order the engine halt and cost
    # ~1.5us each in serialized semaphore handoffs.
    end_blk = nc.main_func.blocks[2]
    for inst in [
        i for i in end_blk.instructions
        if isinstance(i, (mybir.InstDrain, mybir.InstISA))
        or (isinstance(i, mybir.InstEventSemaphore)
            and str(getattr(i, "name", "")).startswith("barrier_"))
    ]:
        end_blk.instructions.remove(inst)

    nc.compile()
    return nc


_NC_CACHE = None


def _get_nc():
    global _NC_CACHE
    if _NC_CACHE is None:
        _NC_CACHE = _build_bass()
    return _NC_CACHE


_CST = _consts()


def make_in_maps(imgs, weight):
    """Per-core input dicts: shuffled channel-0 row slice + weight shard."""
    wr = weight.reshape(OUT, H // BS, WD // BS, NF)  # [o, i_glob, j_glob, m]
    in_maps = []
    for c in range(N_CORES):
        xc = imgs[:, 0, RPC * c:RPC * (c + 1), :]    # [32, 64, 512] f32
        # -> [q4, w128, a8, b32, i8]: element (b, i*8+a, q*128+w)
        xd = xc.reshape(B, IPC, BS, 4, 128).transpose(3, 4, 2, 0, 1)
        xd = np.ascontiguousarray(xd.reshape(4, 128, BS * B * IPC)).astype(NPBF16)
        wc = wr[:, IPC * c:IPC * (c + 1)]            # [o, i, j_glob, m]
        # p = wc2*64 + j16*4 + m (j_glob = fc*32 + wc2*16 + j16), t = fc*8+i
        wtc = wc.reshape(OUT, IPC, 2, 2, 16, NF)     # o, i, fc, wc2, j16, m
        wtc = wtc.transpose(3, 4, 5, 2, 1, 0)        # wc2, j16, m, fc, i, o
        wtc = np.ascontiguousarray(wtc.reshape(128, NT * OUT)).astype(NPBF16)
        in_maps.append({"x": xd, "cst": _CST, "wt": wtc})
    return in_maps


def kernel(imgs_tensors, weight, bias, block_size=8, num_features=4, **_):
    assert int(block_size) == BS and int(num_features) == NF
    imgs = np.ascontiguousarray(np.asarray(imgs_tensors, dtype=np.float32))
    w = np.ascontiguousarray(np.asarray(weight, dtype=np.float32))
    b = np.asarray(bias, dtype=np.float32)
    assert imgs.shape == (B, 3, H, WD) and w.shape == (OUT, H // BS * WD // BS * NF)

    nc = _get_nc()
    res = run_bass_kernel_spmd(nc, make_in_maps(imgs, w), core_ids=list(range(N_CORES)))
    acc = np.zeros((B, OUT), np.float64)
    for r in res.results:
        acc += r["out"].astype(np.float64).reshape(4, B, OUT).sum(axis=0)
    return (acc + b[None, :]).astype(np.float32)
